# revision 16
# baseline (speedup 1.0000x reference)
"""Trainium2 Bass kernel for nn_LogicalReasoningLayer (moe_routing).

Sharding: 8 cores <- (batch b = c//2, seq half = c%2), 1024 tokens each.
K/V exchanged between seq-half pairs via bf16 AllReduce(add); the remote
half is recovered as (sum - local) in one wide vector op per chunk.

Dtype plan (validated vs reference, ~5e-3 rel err):
  fp8e4 + DoubleRow matmuls: router, MoE experts, proj/q/k/v, attention
  scores/denominator/context, out-proj, agg, gate.  bf16: final output
  GEMM, residual stream, LN stats.  fp32: PSUM, softmax reciprocals.
"""

import sys

sys.path.insert(0, "/opt/trn_rl_repo")

import math

import ml_dtypes
import numpy as np

import concourse.bass as bass
import concourse.bacc as bacc
import concourse.tile as tile
from concourse import mybir
from concourse.bass import ts
from concourse.bass_utils import run_bass_kernel_spmd
from concourse.masks import make_identity

P = 128
H = 512
C = H // P          # 4 feature chunks
KP = C // 2         # 2 contraction pair-groups for DoubleRow
T = 1024            # tokens per core
TT = T // P         # 8 token tiles
TC = T // 512       # 2 token chunks (moving dim 512)
O = 6
NH = 4
HD = 128
D = 3
S = 2048
KT = S // P         # 16 key tiles
NPAIR = KT // 2     # 8 key-tile pairs
EPS = 1e-5
F32 = mybir.dt.float32
BF16 = mybir.dt.bfloat16
F8 = mybir.dt.float8e4
AF = mybir.ActivationFunctionType
ALU = mybir.AluOpType
DR = mybir.MatmulPerfMode.DoubleRow
RG = [[0, 1], [2, 3], [4, 5], [6, 7]]
GB = 2              # MoE units per act-table batch group

_CACHE = {}


def bcast_ap(handle, n_free):
    """[n_free] DRAM vector -> [P, n_free] stride-0 partition-broadcast AP."""
    return bass.AP(tensor=handle, offset=0, ap=[[0, P], [1, n_free]])


def build_bass(sim_mode=False):
    nc = bacc.Bacc("TRN2", target_bir_lowering=False, num_devices=8)

    f = F32
    # ---------------- external inputs ----------------
    xT8_in = nc.dram_tensor("xT8", [P, C, T], F8, kind="ExternalInput")
    xTb_in = nc.dram_tensor("xTb", [P, C, T], BF16, kind="ExternalInput")
    onesh_in = nc.dram_tensor("onesh_in", [P, P], BF16, kind="ExternalInput")
    ones8_in = nc.dram_tensor("ones8_in", [P, 2, P], F8, kind="ExternalInput")
    selw1_in = nc.dram_tensor("selw1", [P, C, H], F8, kind="ExternalInput")
    sw2_in = nc.dram_tensor("sw2", [P, C, O], F8, kind="ExternalInput")
    selb1 = nc.dram_tensor("selb1", [P, C], f, kind="ExternalInput")
    selb2 = nc.dram_tensor("selb2", [O], f, kind="ExternalInput")
    w1x_in = nc.dram_tensor("w1x", [O, P, C, H], F8, kind="ExternalInput")
    w2_in = nc.dram_tensor("w2", [O, P, C, H], F8, kind="ExternalInput")
    c1_d = nc.dram_tensor("c1", [P, O, C], f, kind="ExternalInput")
    lng_d = nc.dram_tensor("lng", [P, O, C], f, kind="ExternalInput")
    lnb_d = nc.dram_tensor("lnb", [P, O, C], f, kind="ExternalInput")
    wq_in = nc.dram_tensor("wq", [P, C, H], F8, kind="ExternalInput")
    wk_in = nc.dram_tensor("wk", [P, C, H], F8, kind="ExternalInput")
    wv_in = nc.dram_tensor("wv", [P, C, H], F8, kind="ExternalInput")
    ibq_d = nc.dram_tensor("ibq", [64, NH, 2], f, kind="ExternalInput")
    ibk_d = nc.dram_tensor("ibk", [64, NH, 2], f, kind="ExternalInput")
    ibv_d = nc.dram_tensor("ibv", [H], f, kind="ExternalInput")
    ow_in = nc.dram_tensor("ow", [P, C, H], F8, kind="ExternalInput")
    oba_d = nc.dram_tensor("oba", [P, C], f, kind="ExternalInput")
    rpj_in = nc.dram_tensor("rpj", [P, C, H], F8, kind="ExternalInput")
    rpb_d = nc.dram_tensor("rpb", [P, C], f, kind="ExternalInput")
    wa_in = nc.dram_tensor("wa", [P, C, H], F8, kind="ExternalInput")
    dc_d = nc.dram_tensor("dc", [P, D, C], f, kind="ExternalInput")
    gw1_in = nc.dram_tensor("gw1", [P, C, H], F8, kind="ExternalInput")
    gw2_in = nc.dram_tensor("gw2", [P, C, H], F8, kind="ExternalInput")
    gateb = nc.dram_tensor("gateb", [P, C], f, kind="ExternalInput")
    olng = nc.dram_tensor("olng", [P, C], f, kind="ExternalInput")
    olnb = nc.dram_tensor("olnb", [P, C], f, kind="ExternalInput")
    wf_in = nc.dram_tensor("wf", [P, C, H], BF16, kind="ExternalInput")
    outbF = nc.dram_tensor("outbF", [H], f, kind="ExternalInput")

    out_d = nc.dram_tensor("out", [T, H], f, kind="ExternalOutput")

    # ---------------- DRAM scratch ----------------
    opw_dram = nc.dram_tensor("opw_dram", [O, T], BF16)
    k_send = nc.dram_tensor("k_send", [NH, 64, 2, T], BF16)
    k_sum = nc.dram_tensor("k_sum", [NH, 64, 2, T], BF16)
    v_send = nc.dram_tensor("v_send", [P, TT, H], BF16)
    v_sum = nc.dram_tensor("v_sum", [P, TT, H], BF16)

    scale_s = 1.0 / math.sqrt(HD)

    with tile.TileContext(nc) as tc:
        with (
            tc.tile_pool(name="singles", bufs=1) as sg,
            tc.tile_pool(name="wts", bufs=1) as wp,
            tc.tile_pool(name="act", bufs=1) as ap_,
            tc.tile_pool(name="biga", bufs=1) as bigp,
            tc.tile_pool(name="p8t", bufs=2) as p8p,
            tc.tile_pool(name="pre", bufs=GB) as prp,
            tc.tile_pool(name="hh", bufs=2) as hp,
            tc.tile_pool(name="t512", bufs=2) as tp,
            tc.tile_pool(name="exl", bufs=2) as xp,
            tc.tile_pool(name="kch", bufs=2) as kcp,
            tc.tile_pool(name="vch", bufs=2) as vcp,
            tc.tile_pool(name="wrp", bufs=2) as wrpp,
            tc.tile_pool(name="P1", bufs=2, space="PSUM") as P1,
            tc.tile_pool(name="P2", bufs=2, space="PSUM") as P2,
            tc.tile_pool(name="P3", bufs=2, space="PSUM") as P3,
        ):
            # ---------- constants / weights resident in SBUF ----------
            ident = sg.tile([P, P], f, tag="ident")
            make_identity(nc, ident)
            onesh = sg.tile([P, P], BF16, tag="onesh")
            nc.sync.dma_start(onesh, onesh_in[:])
            ones8 = sg.tile([P, 2, P], F8, tag="ones8")
            nc.sync.dma_start(ones8, ones8_in[:])
            eps_t = sg.tile([P, 1], f, tag="eps")
            nc.vector.memset(eps_t, EPS)

            sb1 = sg.tile([P, C], f, tag="sb1")
            nc.sync.dma_start(sb1, selb1[:])
            sb2b = sg.tile([P, O], f, tag="sb2b")
            nc.sync.dma_start(sb2b, bcast_ap(selb2, O))
            c1s = sg.tile([P, O, C], f, tag="c1s")
            nc.sync.dma_start(c1s, c1_d[:])
            lngs = sg.tile([P, O, C], f, tag="lngs")
            nc.sync.dma_start(lngs, lng_d[:])
            lnbs = sg.tile([P, O, C], f, tag="lnbs")
            nc.sync.dma_start(lnbs, lnb_d[:])
            ibq = sg.tile([64, NH, 2], f, tag="ibq")
            nc.sync.dma_start(ibq, ibq_d[:])
            ibk = sg.tile([64, NH, 2], f, tag="ibk")
            nc.sync.dma_start(ibk, ibk_d[:])
            ibv = sg.tile([P, H], f, tag="ibv")
            nc.sync.dma_start(ibv, bcast_ap(ibv_d, H))
            oba = sg.tile([P, C], f, tag="oba")
            nc.sync.dma_start(oba, oba_d[:])
            rpb = sg.tile([P, C], f, tag="rpb")
            nc.sync.dma_start(rpb, rpb_d[:])
            dcs = sg.tile([P, D, C], f, tag="dcs")
            nc.sync.dma_start(dcs, dc_d[:])
            gbs = sg.tile([P, C], f, tag="gbs")
            nc.sync.dma_start(gbs, gateb[:])
            og = sg.tile([P, C], f, tag="og")
            nc.sync.dma_start(og, olng[:])
            ob = sg.tile([P, C], f, tag="ob")
            nc.sync.dma_start(ob, olnb[:])
            obF = sg.tile([P, H], f, tag="obF")
            nc.sync.dma_start(obF, bcast_ap(outbF, H))

            selw1 = wp.tile([P, C, H], F8, tag="selw1")
            nc.sync.dma_start(selw1, selw1_in[:])
            sw2 = wp.tile([P, C, O], F8, tag="sw2")
            nc.sync.dma_start(sw2, sw2_in[:])
            xT8 = ap_.tile([P, C, T], F8, tag="xT8")
            nc.sync.dma_start(xT8, xT8_in[:])
            xTb = ap_.tile([P, C, T], BF16, tag="xTb")
            nc.sync.dma_start(xTb, xTb_in[:])
            w1s, w2s = [], []
            for o in range(O):
                w1o = wp.tile([P, C, H], F8, tag=f"w1_{o}", name=f"w1o_{o}")
                nc.sync.dma_start(w1o, w1x_in[o])
                w1s.append(w1o)
                w2o = wp.tile([P, C, H], F8, tag=f"w2_{o}", name=f"w2o_{o}")
                nc.sync.dma_start(w2o, w2_in[o])
                w2s.append(w2o)
            wq = wp.tile([P, C, H], F8, tag="wq")
            nc.sync.dma_start(wq, wq_in[:])
            wk = wp.tile([P, C, H], F8, tag="wk")
            nc.sync.dma_start(wk, wk_in[:])
            wv = wp.tile([P, C, H], F8, tag="wv")
            nc.sync.dma_start(wv, wv_in[:])
            ow = wp.tile([P, C, H], F8, tag="ow")
            nc.sync.dma_start(ow, ow_in[:])
            rpj = wp.tile([P, C, H], F8, tag="rpj")
            nc.sync.dma_start(rpj, rpj_in[:])
            wa = wp.tile([P, C, H], F8, tag="wa")
            nc.sync.dma_start(wa, wa_in[:])
            gw1 = wp.tile([P, C, H], F8, tag="gw1")
            nc.sync.dma_start(gw1, gw1_in[:])
            gw2 = wp.tile([P, C, H], F8, tag="gw2")
            nc.sync.dma_start(gw2, gw2_in[:])
            wf = wp.tile([P, C, H], BF16, tag="wf")
            nc.sync.dma_start(wf, wf_in[:])

            # ---------- resident activations ----------
            enhT = ap_.tile([P, C, T], BF16, tag="enhT")
            enh8 = ap_.tile([P, C, T], F8, tag="enh8")
            rec8 = ap_.tile([P, C, T], F8, tag="rec8")
            qT8 = ap_.tile([64, NH, 2, T], F8, tag="qT8")
            kT8 = ap_.tile([64, NH, 2, T], F8, tag="kT8")
            krem8 = ap_.tile([64, NH, 2, T], F8, tag="krem8")
            vloc8 = ap_.tile([P, TT, H], F8, tag="vloc8")
            vrem8 = ap_.tile([P, TT, H], F8, tag="vrem8")
            ctx8 = ap_.tile([P, NH, T], F8, tag="ctx8")
            opwT = ap_.tile([O, T], BF16, tag="opwT")

            def dr_gemm(ps_out, lhsT_full, rhs_full, m, tslice, start=True,
                        stop=True):
                """H-contraction fp8 DoubleRow GEMM tile: out[m-chunk, tslice]."""
                for kp in range(KP):
                    nc.tensor.matmul(
                        ps_out,
                        lhsT_full[:, 2 * kp : 2 * kp + 2, ts(m, P)],
                        rhs_full[:, 2 * kp : 2 * kp + 2, tslice],
                        start=start and (kp == 0),
                        stop=stop and (kp == KP - 1),
                        perf_mode=DR,
                    )

            # ---------- phase 1: router ----------
            hr8 = p8p.tile([P, C, T], F8, tag="p8t", name="hr8")
            for m in range(C):
                for t in range(TC):
                    ps = P2.tile([P, 512], f, tag="p2", name=f"rps_{m}_{t}")
                    dr_gemm(ps, selw1, xT8, m, ts(t, 512))
                    nc.scalar.activation(
                        hr8[:, m, ts(t, 512)], ps, AF.Gelu,
                        bias=sb1[:, m : m + 1], scale=1.0,
                    )
            for i in range(TT):
                ps = P3.tile([P, 512], f, tag="p3", name=f"lgps_{i}")
                for k in range(C):
                    nc.tensor.matmul(
                        ps[:, :O], hr8[:, k, ts(i, P)], sw2[:, k, :],
                        start=(k == 0), stop=(k == C - 1),
                    )
                lg = tp.tile([P, O], f, tag="sm6", name=f"lg_{i}")
                nc.vector.tensor_tensor(lg, ps[:, :O], sb2b, ALU.add)
                ex = tp.tile([P, O], f, tag="sm6b", name=f"ex_{i}")
                s_ = tp.tile([P, 1], f, tag="sm1", name=f"s_{i}")
                nc.scalar.activation(ex, lg, AF.Exp, accum_out=s_)
                nc.vector.reciprocal(s_, s_)
                nc.vector.tensor_scalar_mul(ex, ex, s_)
                tps = P2.tile([P, 512], f, tag="p2", name=f"tps_{i}")
                nc.tensor.transpose(tps[:O, :P], ex, ident)
                nc.vector.tensor_copy(out=opwT[:, ts(i, P)], in_=tps[:O, :P])
            nc.sync.dma_start(opw_dram[:], opwT[:])

            # ---------- phase 2: MoE (6 experts, PSUM-accumulated) ----------
            for t in range(TC):
                acc01 = P1.tile([P, 2, 512], f, tag="p1", name=f"acc01_{t}")
                acc23 = P1.tile([P, 2, 512], f, tag="p1", name=f"acc23_{t}")
                accs = [acc01[:, 0, :], acc01[:, 1, :],
                        acc23[:, 0, :], acc23[:, 1, :]]
                # group-of-GB software pipeline to batch the Sqrt table loads
                for g0 in range(0, O, GB):
                    grp = list(range(g0, min(g0 + GB, O)))
                    pres, rvs, wrs = {}, {}, {}
                    for o in grp:
                        wr = wrpp.tile([P, 512], BF16, tag="wrp",
                                       name=f"wr_{o}_{t}")
                        nc.sync.dma_start(
                            wr,
                            bass.AP(tensor=opw_dram, offset=o * T + t * 512,
                                    ap=[[0, P], [1, 512]]),
                        )
                        wrs[o] = wr
                        pre = prp.tile([P, C, 512], BF16, tag="pre",
                                       name=f"pre_{o}_{t}")
                        for m in range(C):
                            ps = P2.tile([P, 512], f, tag="p2",
                                         name=f"g1ps_{o}_{t}_{m}")
                            dr_gemm(ps, w1s[o], xT8, m, ts(t, 512))
                            nc.vector.tensor_scalar_add(
                                pre[:, m, :], ps, c1s[:, o, m : m + 1]
                            )
                        pres[o] = pre
                        # stats: mean and E[x^2] via 1/H-matmul accumulation
                        psm = P3.tile([P, 512], f, tag="p3",
                                      name=f"psm_{o}_{t}")
                        for c in range(C):
                            nc.tensor.matmul(psm, onesh, pre[:, c, :],
                                             start=(c == 0), stop=(c == C - 1))
                        psq = P3.tile([P, 512], f, tag="p3",
                                      name=f"psq_{o}_{t}")
                        for c in range(C):
                            sq = tp.tile([P, 512], BF16, tag="sq",
                                         name=f"sq_{o}_{t}_{c}")
                            nc.scalar.activation(sq, pre[:, c, :], AF.Square)
                            nc.tensor.matmul(psq, onesh, sq,
                                             start=(c == 0), stop=(c == C - 1))
                        mean = tp.tile([P, 512], BF16, tag="mean",
                                       name=f"mean_{o}_{t}")
                        nc.vector.tensor_copy(out=mean, in_=psm)
                        sqm = tp.tile([P, 512], BF16, tag="sq2",
                                      name=f"sqm_{o}_{t}")
                        nc.scalar.activation(sqm, psq, AF.Copy)
                        m2v = tp.tile([P, 512], BF16, tag="lnstat_f",
                                      name=f"m2v_{o}_{t}")
                        nc.vector.tensor_tensor(m2v, mean, mean, ALU.mult)
                        nc.vector.tensor_tensor(m2v, sqm, m2v, ALU.subtract)
                        nc.vector.tensor_scalar_add(m2v, m2v, EPS)
                        rv = tp.tile([P, 512], f, tag="rvf",
                                     name=f"rv_{o}_{t}")
                        nc.vector.reciprocal(rv, m2v)
                        rvs[o] = (mean, rv)
                    # batched Sqrt ops (one act-table load per group)
                    rstds = {}
                    for o in grp:
                        rstd = tp.tile([P, 512], BF16, tag="rstd",
                                       name=f"rstd_{o}_{t}")
                        nc.scalar.activation(rstd, rvs[o][1], AF.Sqrt)
                        rstds[o] = rstd
                    # normalize + gelu (batched) + weight + GEMM2
                    for o in grp:
                        pre = pres[o]
                        mean = rvs[o][0]
                        rstd = rstds[o]
                        h = hp.tile([P, C, 512], BF16, tag="h",
                                    name=f"h_{o}_{t}")
                        for m in range(C):
                            d1 = tp.tile([P, 512], BF16, tag="dn",
                                         name=f"d1_{o}_{t}_{m}")
                            nc.vector.tensor_tensor(
                                d1, pre[:, m, :], mean, ALU.subtract
                            )
                            nc.vector.tensor_tensor(d1, d1, rstd, ALU.mult)
                            nc.scalar.activation(
                                h[:, m, :], d1, AF.Gelu,
                                bias=lnbs[:, o, m : m + 1],
                                scale=lngs[:, o, m : m + 1],
                            )
                        h8 = hp.tile([P, C, 512], F8, tag="h8",
                                     name=f"h8_{o}_{t}")
                        for m in range(C):
                            eng = nc.vector if m % 2 == 0 else nc.gpsimd
                            eng.tensor_tensor(
                                h8[:, m, :], h[:, m, :], wrs[o], ALU.mult
                            )
                        for m in range(C):
                            dr_gemm(accs[m], w2s[o], h8, m, slice(None),
                                    start=(o == 0), stop=(o == O - 1))
                # enhanced = x + routed expert sum
                for m in range(C):
                    nc.vector.tensor_tensor(
                        enhT[:, m, ts(t, 512)], accs[m], xTb[:, m, ts(t, 512)],
                        ALU.add,
                    )
                for m in range(C):
                    nc.gpsimd.tensor_copy(
                        out=enh8[:, m, ts(t, 512)], in_=enhT[:, m, ts(t, 512)]
                    )

            # ---------- phase 3: recursive reasoning (3 depths) ----------
            for d in range(D):
                src8 = enh8 if d == 0 else rec8
                proj8 = p8p.tile([P, C, T], F8, tag="p8t", name=f"proj8_{d}")
                for m in range(C):
                    for t in range(TC):
                        ps = P2.tile([P, 512], f, tag="p2",
                                     name=f"pjps_{d}_{m}_{t}")
                        dr_gemm(ps, rpj, src8, m, ts(t, 512))
                        nc.vector.tensor_scalar_add(
                            proj8[:, m, ts(t, 512)], ps, rpb[:, m : m + 1]
                        )
                # V first (feeds all heads' remote half), then per-head
                # K chain -> Q -> attention, so exp(h) overlaps K/Q(h+1).
                for qtr in range(4):
                    vc = vcp.tile([P, 2, H], BF16, tag="vch",
                                  name=f"vc_{d}_{qtr}")
                    for ii in range(2):
                        i = qtr * 2 + ii
                        ps = P2.tile([P, 512], f, tag="p2",
                                     name=f"vps_{d}_{i}")
                        for kp in range(KP):
                            nc.tensor.matmul(
                                ps,
                                proj8[:, 2 * kp : 2 * kp + 2, ts(i, P)],
                                wv[:, 2 * kp : 2 * kp + 2, :],
                                start=(kp == 0), stop=(kp == KP - 1),
                                perf_mode=DR,
                            )
                        nc.vector.tensor_tensor(vc[:, ii, :], ps, ibv,
                                                ALU.add)
                        nc.gpsimd.tensor_copy(out=vloc8[:, i, :],
                                              in_=vc[:, ii, :])
                    nc.sync.dma_start(v_send[:, qtr * 2 : qtr * 2 + 2], vc)
                if sim_mode:
                    nc.sync.dma_start(v_sum[:], v_send[:])
                else:
                    nc.gpsimd.collective_compute(
                        "AllReduce", ALU.add, replica_groups=RG,
                        ins=[v_send[:]], outs=[v_sum[:]],
                    )
                for qtr in range(4):
                    sl = (slice(None), slice(qtr * 2, qtr * 2 + 2),
                          slice(None))
                    vsm = vcp.tile([P, 2, H], BF16, tag="vch2",
                                   name=f"vsm_{d}_{qtr}")
                    nc.sync.dma_start(vsm, v_sum[:, qtr * 2 : qtr * 2 + 2])
                    nc.gpsimd.tensor_tensor(vrem8[sl], vsm, vloc8[sl],
                                            ALU.subtract)
                for h in range(NH):
                    # K(h): GEMM -> bf16 chunk -> DMA; per-head AllReduce
                    for t in range(TC):
                        ps = P1.tile([64, 2, 512], f, tag="p1",
                                     name=f"kps_{d}_{h}_{t}")
                        for s in range(2):
                            for kp in range(KP):
                                nc.tensor.matmul(
                                    ps[:, s, :],
                                    wk[:, 2 * kp : 2 * kp + 2,
                                       (h * 2 + s) * 64 : (h * 2 + s + 1) * 64],
                                    proj8[:, 2 * kp : 2 * kp + 2, ts(t, 512)],
                                    start=(kp == 0), stop=(kp == KP - 1),
                                    perf_mode=DR,
                                )
                        kc = kcp.tile([64, 2, 512], BF16, tag="kch",
                                      name=f"kc_{d}_{h}_{t}")
                        for s in range(2):
                            nc.vector.tensor_scalar_add(
                                kc[:, s, :], ps[:, s, :], ibk[:, h, s : s + 1]
                            )
                        nc.gpsimd.tensor_copy(
                            out=kT8[:, h, :, ts(t, 512)], in_=kc
                        )
                        nc.sync.dma_start(k_send[h, :, :, ts(t, 512)], kc)
                    if sim_mode:
                        nc.sync.dma_start(k_sum[h], k_send[h])
                    else:
                        nc.gpsimd.collective_compute(
                            "AllReduce", ALU.add, replica_groups=RG,
                            ins=[k_send[h]], outs=[k_sum[h]],
                        )
                    for t in range(TC):
                        ksm = kcp.tile([64, 2, 512], BF16, tag="ksm",
                                       name=f"ksm_{d}_{h}_{t}")
                        nc.sync.dma_start(ksm, k_sum[h, :, :, ts(t, 512)])
                        nc.gpsimd.tensor_tensor(
                            krem8[:, h, :, ts(t, 512)], ksm,
                            kT8[:, h, :, ts(t, 512)], ALU.subtract,
                        )
                    # Q(h)
                    for t in range(TC):
                        ps = P1.tile([64, 2, 512], f, tag="p1",
                                     name=f"qps_{d}_{h}_{t}")
                        for s in range(2):
                            for kp in range(KP):
                                nc.tensor.matmul(
                                    ps[:, s, :],
                                    wq[:, 2 * kp : 2 * kp + 2,
                                       (h * 2 + s) * 64 : (h * 2 + s + 1) * 64],
                                    proj8[:, 2 * kp : 2 * kp + 2, ts(t, 512)],
                                    start=(kp == 0), stop=(kp == KP - 1),
                                    perf_mode=DR,
                                )
                        for s in range(2):
                            nc.vector.tensor_scalar_add(
                                qT8[:, h, s, ts(t, 512)], ps[:, s, :],
                                ibq[:, h, s : s + 1],
                            )
                    # attention(h)
                    for qc in range(TC):
                        dacc = P2.tile([P, 512], f, tag="p2",
                                       name=f"dacc_{d}_{h}_{qc}")
                        cctx = P3.tile([P, 512], f, tag="p3",
                                       name=f"cctx_{d}_{h}_{qc}")
                        for j in range(NPAIR):
                            sp = P1.tile([P, 2, 512], f, tag="p1",
                                         name=f"sp_{d}_{h}_{qc}_{j}")
                            for u in range(2):
                                kt = 2 * j + u
                                if kt < KT // 2:
                                    klhs = kT8[:, h, :, ts(kt, P)]
                                else:
                                    klhs = krem8[:, h, :, ts(kt - KT // 2, P)]
                                nc.tensor.matmul(
                                    sp[:, u, :], klhs,
                                    qT8[:, h, :, ts(qc, 512)],
                                    start=True, stop=True, perf_mode=DR,
                                )
                            ex8 = xp.tile([P, 2, 512], F8, tag="exl",
                                          name=f"ex_{d}_{h}_{qc}_{j}")
                            nc.scalar.activation(ex8, sp, AF.Exp,
                                                 scale=scale_s)
                            if j < NPAIR // 2:
                                vsel = vloc8[:, 2 * j : 2 * j + 2, ts(h, P)]
                            else:
                                jj = 2 * j - TT
                                vsel = vrem8[:, jj : jj + 2, ts(h, P)]
                            nc.tensor.matmul(dacc, ones8, ex8,
                                             start=(j == 0),
                                             stop=(j == NPAIR - 1),
                                             perf_mode=DR)
                            nc.tensor.matmul(cctx, vsel, ex8,
                                             start=(j == 0),
                                             stop=(j == NPAIR - 1),
                                             perf_mode=DR)
                        rd = tp.tile([P, 512], f, tag="rvf",
                                     name=f"rd_{d}_{h}_{qc}")
                        nc.vector.reciprocal(rd, dacc)
                        nc.vector.tensor_tensor(
                            ctx8[:, h, ts(qc, 512)], cctx, rd, ALU.mult
                        )
                # out-proj + aggregate
                att8 = p8p.tile([P, C, T], F8, tag="p8t", name=f"att8_{d}")
                for m in range(C):
                    for t in range(TC):
                        ps = P2.tile([P, 512], f, tag="p2",
                                     name=f"ops_{d}_{m}_{t}")
                        dr_gemm(ps, ow, ctx8, m, ts(t, 512))
                        nc.vector.tensor_scalar_add(
                            att8[:, m, ts(t, 512)], ps, oba[:, m : m + 1]
                        )
                for m in range(C):
                    for t in range(TC):
                        ps = P3.tile([P, 512], f, tag="p3",
                                     name=f"agps_{d}_{m}_{t}")
                        dr_gemm(ps, wa, att8, m, ts(t, 512))
                        nc.vector.tensor_scalar_add(
                            rec8[:, m, ts(t, 512)], ps, dcs[:, d, m : m + 1]
                        )
                sc = 0.5 ** (d + 1)
                for m in range(C):
                    for t in range(TC):
                        tmp = tp.tile([P, 512], BF16, tag="t512",
                                      name=f"et_{d}_{m}_{t}")
                        nc.vector.tensor_scalar_mul(
                            tmp, rec8[:, m, ts(t, 512)], sc
                        )
                        nc.vector.tensor_tensor(
                            enhT[:, m, ts(t, 512)], enhT[:, m, ts(t, 512)],
                            tmp, ALU.add,
                        )

            # ---------- phase 4: gating ----------
            for m in range(C):
                for t in range(TC):
                    nc.gpsimd.tensor_copy(
                        out=enh8[:, m, ts(t, 512)], in_=enhT[:, m, ts(t, 512)]
                    )
            gateT = bigp.tile([P, C, T], BF16, tag="biga", name="gateT")
            for t in range(TC):
                for mp_ in range(2):
                    gps = P1.tile([P, 2, 512], f, tag="p1",
                                  name=f"gps_{t}_{mp_}")
                    for u in range(2):
                        m = 2 * mp_ + u
                        for kp in range(KP):
                            nc.tensor.matmul(
                                gps[:, u, :],
                                gw1[:, 2 * kp : 2 * kp + 2, ts(m, P)],
                                xT8[:, 2 * kp : 2 * kp + 2, ts(t, 512)],
                                start=(kp == 0), stop=False, perf_mode=DR,
                            )
                        for kp in range(KP):
                            nc.tensor.matmul(
                                gps[:, u, :],
                                gw2[:, 2 * kp : 2 * kp + 2, ts(m, P)],
                                enh8[:, 2 * kp : 2 * kp + 2, ts(t, 512)],
                                start=False, stop=(kp == KP - 1),
                                perf_mode=DR,
                            )
                    # one wide sigmoid per m-pair; bias varies per m chunk so
                    # add it with the per-partition scalar slots of the two
                    # halves separately only if needed (biases share partition
                    # layout across the pair -> same [P,1] slice per half).
                    for u in range(2):
                        m = 2 * mp_ + u
                        nc.scalar.activation(
                            gateT[:, m, ts(t, 512)], gps[:, u, :], AF.Sigmoid,
                            bias=gbs[:, m : m + 1], scale=1.0,
                        )
            for m in range(C):
                for t in range(TC):
                    sl = (slice(None), m, ts(t, 512))
                    dt_ = tp.tile([P, 512], BF16, tag="t512",
                                  name=f"gd_{m}_{t}")
                    nc.vector.tensor_tensor(dt_, enhT[sl], xTb[sl],
                                            ALU.subtract)
                    nc.vector.tensor_tensor(dt_, dt_, gateT[sl], ALU.mult)
                    nc.vector.tensor_tensor(enhT[sl], xTb[sl], dt_, ALU.add)

            # ---------- phase 5: output LayerNorm + final linear ----------
            lnT = bigp.tile([P, C, T], BF16, tag="biga", name="lnT")
            stats = {}
            for t in range(TC):
                psm = P2.tile([P, 512], f, tag="p2", name=f"fpsm_{t}")
                for c in range(C):
                    nc.tensor.matmul(psm, onesh, enhT[:, c, ts(t, 512)],
                                     start=(c == 0), stop=(c == C - 1))
                psq = P3.tile([P, 512], f, tag="p3", name=f"fpsq_{t}")
                for c in range(C):
                    sq = tp.tile([P, 512], BF16, tag="sq",
                                 name=f"fsq_{t}_{c}")
                    nc.scalar.activation(sq, enhT[:, c, ts(t, 512)],
                                         AF.Square)
                    nc.tensor.matmul(psq, onesh, sq,
                                     start=(c == 0), stop=(c == C - 1))
                mean = tp.tile([P, 512], BF16, tag="mean", name=f"fmean_{t}")
                nc.vector.tensor_copy(out=mean, in_=psm)
                sqm = tp.tile([P, 512], BF16, tag="sq2", name=f"fsqm_{t}")
                nc.scalar.activation(sqm, psq, AF.Copy)
                m2v = tp.tile([P, 512], BF16, tag="lnstat_f", name=f"fm2v_{t}")
                nc.vector.tensor_tensor(m2v, mean, mean, ALU.mult)
                nc.vector.tensor_tensor(m2v, sqm, m2v, ALU.subtract)
                nc.vector.tensor_scalar_add(m2v, m2v, EPS)
                rv = tp.tile([P, 512], f, tag="rvf", name=f"frv_{t}")
                nc.vector.reciprocal(rv, m2v)
                stats[t] = (mean, rv)
            for t in range(TC):
                mean, rv = stats[t]
                rstd = tp.tile([P, 512], BF16, tag="rstd", name=f"frstd_{t}")
                nc.scalar.activation(rstd, rv, AF.Sqrt)
                stats[t] = (mean, rstd)
            for t in range(TC):
                mean, rstd = stats[t]
                for m in range(C):
                    sl = (slice(None), m, ts(t, 512))
                    d1 = tp.tile([P, 512], BF16, tag="dn",
                                 name=f"fd1_{t}_{m}")
                    nc.vector.tensor_tensor(d1, enhT[sl], mean, ALU.subtract)
                    nc.vector.tensor_tensor(d1, d1, rstd, ALU.mult)
                    nc.vector.tensor_scalar(
                        lnT[sl], d1, og[:, m : m + 1], ob[:, m : m + 1],
                        ALU.mult, ALU.add,
                    )
                for i in range(t * TT // TC, (t + 1) * TT // TC):
                    ps = P2.tile([P, 512], f, tag="p2", name=f"fps_{i}")
                    for k in range(C):
                        nc.tensor.matmul(
                            ps, lnT[:, k, ts(i, P)], wf[:, k, :],
                            start=(k == 0), stop=(k == C - 1),
                        )
                    ot = tp.tile([P, 512], f, tag="t512", name=f"ot_{i}")
                    nc.vector.tensor_tensor(ot, ps, obF, ALU.add)
                    nc.sync.dma_start(out_d[ts(i, P), :], ot)

    nc.compile()
    return nc


# ---------------------------------------------------------------------------
# host side
# ---------------------------------------------------------------------------

FP8 = ml_dtypes.float8_e4m3
BFH = ml_dtypes.bfloat16


def _lhsT(w):
    """w [fout, fin] (y = x @ w.T) -> stationary layout [P, fin//P, fout]."""
    wt = np.ascontiguousarray(np.asarray(w, np.float32).T)
    fi, fo = wt.shape
    return np.ascontiguousarray(wt.reshape(fi // P, P, fo).transpose(1, 0, 2))


def _fvec(v, nch=None):
    v = np.asarray(v, np.float32)
    n = v.shape[-1] // P if nch is None else nch
    return np.ascontiguousarray(v.reshape(n, P).T)


def _prep_weights(i):
    w = {}
    w["selw1"] = _lhsT(i["sel_W1"]).astype(FP8)
    w["sw2"] = _lhsT(i["sel_W2"]).astype(FP8)
    w["selb1"] = _fvec(i["sel_b1"])
    w["selb2"] = np.asarray(i["sel_b2"], np.float32)
    w["w1x"] = np.stack(
        [_lhsT(i["op_W1"][o, :, :H]) for o in range(O)]
    ).astype(FP8)
    w["w2"] = np.stack([_lhsT(i["op_W2"][o]) for o in range(O)]).astype(FP8)
    c1 = np.stack(
        [i["op_emb"][o] @ i["op_W1"][o, :, H:].T + i["op_b1"][o]
         for o in range(O)]
    ).astype(np.float32)
    w["c1"] = np.ascontiguousarray(
        np.stack([_fvec(c1[o]) for o in range(O)]).transpose(1, 0, 2)
    )
    w["lng"] = np.ascontiguousarray(
        np.stack([_fvec(i["op_ln_g"][o]) for o in range(O)]).transpose(1, 0, 2)
    )
    w["lnb"] = np.ascontiguousarray(
        np.stack([_fvec(i["op_ln_b"][o]) for o in range(O)]).transpose(1, 0, 2)
    )
    w["wq"] = _lhsT(i["attn_in_w"][:H]).astype(FP8)
    w["wk"] = _lhsT(i["attn_in_w"][H : 2 * H]).astype(FP8)
    w["wv"] = _lhsT(i["attn_in_w"][2 * H :]).astype(FP8)
    w["ibq"] = np.ascontiguousarray(
        np.asarray(i["attn_in_b"][:H], np.float32)
        .reshape(NH, 2, 64).transpose(2, 0, 1)
    )
    w["ibk"] = np.ascontiguousarray(
        np.asarray(i["attn_in_b"][H : 2 * H], np.float32)
        .reshape(NH, 2, 64).transpose(2, 0, 1)
    )
    w["ibv"] = np.asarray(i["attn_in_b"][2 * H :], np.float32)
    w["ow"] = _lhsT(i["attn_out_w"]).astype(FP8)
    w["oba"] = _fvec(i["attn_out_b"])
    w["rpj"] = _lhsT(i["rec_proj_w"]).astype(FP8)
    w["rpb"] = _fvec(i["rec_proj_b"])
    w["wa"] = _lhsT(i["rec_agg_w"][:, :H]).astype(FP8)
    dc = np.stack(
        [i["depth_emb"][d] @ i["rec_agg_w"][:, H:].T + i["rec_agg_b"]
         for d in range(D)]
    ).astype(np.float32)
    w["dc"] = np.ascontiguousarray(
        np.stack([_fvec(dc[d]) for d in range(D)]).transpose(1, 0, 2)
    )
    w["gw1"] = _lhsT(i["gate_w"][:, :H]).astype(FP8)
    w["gw2"] = _lhsT(i["gate_w"][:, H:]).astype(FP8)
    w["gateb"] = _fvec(i["gate_b"])
    w["olng"] = _fvec(i["out_ln_g"])
    w["olnb"] = _fvec(i["out_ln_b"])
    w["wf"] = _lhsT(i["out_w"]).astype(BFH)
    w["outbF"] = np.asarray(i["out_b"], np.float32)
    w["onesh_in"] = np.full((P, P), 1.0 / H, BFH)
    w["ones8_in"] = np.ones((P, 2, P), FP8)
    return w


def make_in_maps(inputs):
    inputs = {k: np.asarray(v, np.float32) for k, v in inputs.items()}
    hs = inputs["hidden_states"]
    w = _prep_weights(inputs)
    in_maps = []
    for c in range(8):
        b, half = c // 2, c % 2
        m = dict(w)
        x = hs[b, half * T : (half + 1) * T, :]       # [T, H]
        xt = np.ascontiguousarray(
            x.T.reshape(C, P, T).transpose(1, 0, 2)   # [P, C, T]
        )
        m["xT8"] = xt.astype(FP8)
        m["xTb"] = xt.astype(BFH)
        in_maps.append(m)
    return in_maps


def assemble_out(results):
    out = np.empty((4, S, H), np.float32)
    for c in range(8):
        b, half = c // 2, c % 2
        out[b, half * T : (half + 1) * T, :] = results[c]["out"]
    return out


def kernel(**inputs):
    in_maps = make_in_maps(inputs)
    if "nc" not in _CACHE:
        _CACHE["nc"] = build_bass()
    res = run_bass_kernel_spmd(nc=_CACHE["nc"], in_maps=in_maps,
                               core_ids=list(range(8)))
    return assemble_out(res.results)


if __name__ == "__main__":
    print("build-only smoke test")
    build_bass()
    print("ok")


# revision 23
# speedup vs baseline: 1.2084x; 1.2084x over previous
"""Trainium2 Bass kernel for nn_LogicalReasoningLayer (moe_routing).

Sharding: 8 cores <- (batch b = c//2, seq half = c%2), 1024 tokens each.
K/V exchanged between seq-half pairs via bf16 AllReduce(add); the remote
half is recovered as (sum - local) in one wide vector op per chunk.

Dtype plan (validated vs reference, ~5e-3 rel err):
  fp8e4 + DoubleRow matmuls: router, MoE experts, proj/q/k/v, attention
  scores/denominator/context, out-proj, agg, gate.  bf16: final output
  GEMM, residual stream, LN stats.  fp32: PSUM, softmax reciprocals.
"""

import sys

sys.path.insert(0, "/opt/trn_rl_repo")

import math

import ml_dtypes
import numpy as np

import concourse.bass as bass
import concourse.bacc as bacc
import concourse.tile as tile
from concourse import mybir
from concourse.bass import ts
from concourse.bass_utils import run_bass_kernel_spmd
from concourse.masks import make_identity

P = 128
H = 512
C = H // P          # 4 feature chunks
KP = C // 2         # 2 contraction pair-groups for DoubleRow
T = 1024            # tokens per core
TT = T // P         # 8 token tiles
TC = T // 512       # 2 token chunks (moving dim 512)
O = 6
NH = 4
HD = 128
D = 3
S = 2048
KT = S // P         # 16 key tiles
NPAIR = KT // 2     # 8 key-tile pairs
EPS = 1e-5
F32 = mybir.dt.float32
BF16 = mybir.dt.bfloat16
F8 = mybir.dt.float8e4
AF = mybir.ActivationFunctionType
ALU = mybir.AluOpType
DR = mybir.MatmulPerfMode.DoubleRow
RG = [[0, 1], [2, 3], [4, 5], [6, 7]]
GB = 3              # MoE units per act-table batch group

_CACHE = {}


def bcast_ap(handle, n_free):
    """[n_free] DRAM vector -> [P, n_free] stride-0 partition-broadcast AP."""
    return bass.AP(tensor=handle, offset=0, ap=[[0, P], [1, n_free]])


def build_bass(sim_mode=False):
    nc = bacc.Bacc("TRN2", target_bir_lowering=False, num_devices=8)

    f = F32
    # ---------------- external inputs ----------------
    xT8_in = nc.dram_tensor("xT8", [P, C, T], F8, kind="ExternalInput")
    xTb_in = nc.dram_tensor("xTb", [P, C, T], BF16, kind="ExternalInput")
    onesh_in = nc.dram_tensor("onesh_in", [P, P], BF16, kind="ExternalInput")
    ones8_in = nc.dram_tensor("ones8_in", [P, 2, P], F8, kind="ExternalInput")
    selw1_in = nc.dram_tensor("selw1", [P, C, H], F8, kind="ExternalInput")
    sw2_in = nc.dram_tensor("sw2", [P, C, O], F8, kind="ExternalInput")
    selb1 = nc.dram_tensor("selb1", [P, C], f, kind="ExternalInput")
    selb2 = nc.dram_tensor("selb2", [O], f, kind="ExternalInput")
    w1x_in = nc.dram_tensor("w1x", [O, P, C, H], F8, kind="ExternalInput")
    w2_in = nc.dram_tensor("w2", [O, P, C, H], F8, kind="ExternalInput")
    c1_d = nc.dram_tensor("c1", [P, O, C], f, kind="ExternalInput")
    lng_d = nc.dram_tensor("lng", [P, O, C], f, kind="ExternalInput")
    lnb_d = nc.dram_tensor("lnb", [P, O, C], f, kind="ExternalInput")
    wq_in = nc.dram_tensor("wq", [P, C, H], F8, kind="ExternalInput")
    wk_in = nc.dram_tensor("wk", [P, C, H], F8, kind="ExternalInput")
    wv_in = nc.dram_tensor("wv", [P, C, H], F8, kind="ExternalInput")
    ibq_d = nc.dram_tensor("ibq", [64, NH, 2], f, kind="ExternalInput")
    ibk_d = nc.dram_tensor("ibk", [64, NH, 2], f, kind="ExternalInput")
    ibv_d = nc.dram_tensor("ibv", [H], f, kind="ExternalInput")
    ow_in = nc.dram_tensor("ow", [P, C, H], F8, kind="ExternalInput")
    oba_d = nc.dram_tensor("oba", [P, C], f, kind="ExternalInput")
    rpj_in = nc.dram_tensor("rpj", [P, C, H], F8, kind="ExternalInput")
    rpb_d = nc.dram_tensor("rpb", [P, C], f, kind="ExternalInput")
    wa_in = nc.dram_tensor("wa", [P, C, H], F8, kind="ExternalInput")
    dc_d = nc.dram_tensor("dc", [P, D, C], f, kind="ExternalInput")
    gw1_in = nc.dram_tensor("gw1", [P, C, H], F8, kind="ExternalInput")
    gw2_in = nc.dram_tensor("gw2", [P, C, H], F8, kind="ExternalInput")
    gateb = nc.dram_tensor("gateb", [P, C], f, kind="ExternalInput")
    olng = nc.dram_tensor("olng", [P, C], f, kind="ExternalInput")
    olnb = nc.dram_tensor("olnb", [P, C], f, kind="ExternalInput")
    wf_in = nc.dram_tensor("wf", [P, C, H], BF16, kind="ExternalInput")
    outbF = nc.dram_tensor("outbF", [H], f, kind="ExternalInput")

    out_d = nc.dram_tensor("out", [T, H], f, kind="ExternalOutput")

    # ---------------- DRAM scratch ----------------
    opw_dram = nc.dram_tensor("opw_dram", [O, T], BF16)
    k_send = nc.dram_tensor("k_send", [NH, 64, 2, T], BF16)
    k_sum = nc.dram_tensor("k_sum", [NH, 64, 2, T], BF16)
    v_send = nc.dram_tensor("v_send", [P, TT, H], BF16)
    v_sum = nc.dram_tensor("v_sum", [P, TT, H], BF16)

    scale_s = 1.0 / math.sqrt(HD)

    with tile.TileContext(nc) as tc:
        with (
            tc.tile_pool(name="singles", bufs=1) as sg,
            tc.tile_pool(name="wts", bufs=1) as wp,
            tc.tile_pool(name="act", bufs=1) as ap_,
            tc.tile_pool(name="biga", bufs=1) as bigp,
            tc.tile_pool(name="p8t", bufs=2) as p8p,
            tc.tile_pool(name="pre", bufs=GB) as prp,
            tc.tile_pool(name="hh", bufs=2) as hp,
            tc.tile_pool(name="t512", bufs=2) as tp,
            tc.tile_pool(name="exl", bufs=2) as xp,
            tc.tile_pool(name="kch", bufs=2) as kcp,
            tc.tile_pool(name="vch", bufs=2) as vcp,
            tc.tile_pool(name="wrp", bufs=2) as wrpp,
            tc.tile_pool(name="P1", bufs=2, space="PSUM") as P1,
            tc.tile_pool(name="P2", bufs=2, space="PSUM") as P2,
            tc.tile_pool(name="P3", bufs=2, space="PSUM") as P3,
        ):
            # ---------- constants / weights resident in SBUF ----------
            ident = sg.tile([P, P], f, tag="ident")
            make_identity(nc, ident)
            onesh = sg.tile([P, P], BF16, tag="onesh")
            nc.sync.dma_start(onesh, onesh_in[:])
            ones8 = sg.tile([P, 2, P], F8, tag="ones8")
            nc.sync.dma_start(ones8, ones8_in[:])
            eps_t = sg.tile([P, 1], f, tag="eps")
            nc.vector.memset(eps_t, EPS)

            selw1 = wp.tile([P, C, H], F8, tag="selw1")
            nc.sync.dma_start(selw1, selw1_in[:])
            xT8 = ap_.tile([P, C, T], F8, tag="xT8")
            nc.sync.dma_start(xT8, xT8_in[:])
            sw2 = wp.tile([P, C, O], F8, tag="sw2")
            nc.sync.dma_start(sw2, sw2_in[:])
            sb1 = sg.tile([P, C], f, tag="sb1")
            nc.sync.dma_start(sb1, selb1[:])
            sb2b = sg.tile([P, O], f, tag="sb2b")
            nc.sync.dma_start(sb2b, bcast_ap(selb2, O))
            c1s = sg.tile([P, O, C], f, tag="c1s")
            nc.sync.dma_start(c1s, c1_d[:])
            lngs = sg.tile([P, O, C], f, tag="lngs")
            nc.sync.dma_start(lngs, lng_d[:])
            lnbs = sg.tile([P, O, C], f, tag="lnbs")
            nc.sync.dma_start(lnbs, lnb_d[:])
            ibq = sg.tile([64, NH, 2], f, tag="ibq")
            nc.sync.dma_start(ibq, ibq_d[:])
            ibk = sg.tile([64, NH, 2], f, tag="ibk")
            nc.sync.dma_start(ibk, ibk_d[:])
            ibv = sg.tile([P, H], f, tag="ibv")
            nc.sync.dma_start(ibv, bcast_ap(ibv_d, H))
            oba = sg.tile([P, C], f, tag="oba")
            nc.sync.dma_start(oba, oba_d[:])
            rpb = sg.tile([P, C], f, tag="rpb")
            nc.sync.dma_start(rpb, rpb_d[:])
            dcs = sg.tile([P, D, C], f, tag="dcs")
            nc.sync.dma_start(dcs, dc_d[:])
            gbs = sg.tile([P, C], f, tag="gbs")
            nc.sync.dma_start(gbs, gateb[:])
            og = sg.tile([P, C], f, tag="og")
            nc.sync.dma_start(og, olng[:])
            ob = sg.tile([P, C], f, tag="ob")
            nc.sync.dma_start(ob, olnb[:])
            obF = sg.tile([P, H], f, tag="obF")
            nc.sync.dma_start(obF, bcast_ap(outbF, H))

            xTb = ap_.tile([P, C, T], BF16, tag="xTb")
            nc.sync.dma_start(xTb, xTb_in[:])
            w1s, w2s = [], []
            for o in range(O):
                w1o = wp.tile([P, C, H], F8, tag=f"w1_{o}", name=f"w1o_{o}")
                nc.sync.dma_start(w1o, w1x_in[o])
                w1s.append(w1o)
                w2o = wp.tile([P, C, H], F8, tag=f"w2_{o}", name=f"w2o_{o}")
                nc.sync.dma_start(w2o, w2_in[o])
                w2s.append(w2o)
            wq = wp.tile([P, C, H], F8, tag="wq")
            nc.sync.dma_start(wq, wq_in[:])
            wk = wp.tile([P, C, H], F8, tag="wk")
            nc.sync.dma_start(wk, wk_in[:])
            wv = wp.tile([P, C, H], F8, tag="wv")
            nc.sync.dma_start(wv, wv_in[:])
            ow = wp.tile([P, C, H], F8, tag="ow")
            nc.sync.dma_start(ow, ow_in[:])
            rpj = wp.tile([P, C, H], F8, tag="rpj")
            nc.sync.dma_start(rpj, rpj_in[:])
            wa = wp.tile([P, C, H], F8, tag="wa")
            nc.sync.dma_start(wa, wa_in[:])
            gw1 = wp.tile([P, C, H], F8, tag="gw1")
            nc.sync.dma_start(gw1, gw1_in[:])
            gw2 = wp.tile([P, C, H], F8, tag="gw2")
            nc.sync.dma_start(gw2, gw2_in[:])
            wf = wp.tile([P, C, H], BF16, tag="wf")
            nc.sync.dma_start(wf, wf_in[:])

            # ---------- resident activations ----------
            enhT = ap_.tile([P, C, T], BF16, tag="enhT")
            enh8 = ap_.tile([P, C, T], F8, tag="enh8")
            rec8 = ap_.tile([P, C, T], F8, tag="rec8")
            qT8 = ap_.tile([64, NH, 2, T], F8, tag="qT8")
            kT8 = ap_.tile([64, NH, 2, T], F8, tag="kT8")
            krem8 = ap_.tile([64, NH, 2, T], F8, tag="krem8")
            vloc8 = ap_.tile([P, TT, H], F8, tag="vloc8")
            vrem8 = ap_.tile([P, TT, H], F8, tag="vrem8")
            ctx8 = ap_.tile([P, NH, T], F8, tag="ctx8")
            opwT = ap_.tile([O, T], BF16, tag="opwT")

            def dr_gemm(ps_out, lhsT_full, rhs_full, m, tslice, start=True,
                        stop=True):
                """H-contraction fp8 DoubleRow GEMM tile: out[m-chunk, tslice]."""
                for kp in range(KP):
                    nc.tensor.matmul(
                        ps_out,
                        lhsT_full[:, 2 * kp : 2 * kp + 2, ts(m, P)],
                        rhs_full[:, 2 * kp : 2 * kp + 2, tslice],
                        start=start and (kp == 0),
                        stop=stop and (kp == KP - 1),
                        perf_mode=DR,
                    )

            # ---------- phase 1: router ----------
            hr8 = p8p.tile([P, C, T], F8, tag="p8t", name="hr8")
            for m in range(C):
                for t in range(TC):
                    ps = P2.tile([P, 512], f, tag="p2", name=f"rps_{m}_{t}")
                    dr_gemm(ps, selw1, xT8, m, ts(t, 512))
                    nc.scalar.activation(
                        hr8[:, m, ts(t, 512)], ps, AF.Gelu,
                        bias=sb1[:, m : m + 1], scale=1.0,
                    )
            for i in range(TT):
                ps = P3.tile([P, 512], f, tag="p3", name=f"lgps_{i}")
                for k in range(C):
                    nc.tensor.matmul(
                        ps[:, :O], hr8[:, k, ts(i, P)], sw2[:, k, :],
                        start=(k == 0), stop=(k == C - 1),
                    )
                lg = tp.tile([P, O], f, tag="sm6", name=f"lg_{i}")
                nc.vector.tensor_tensor(lg, ps[:, :O], sb2b, ALU.add)
                ex = tp.tile([P, O], f, tag="sm6b", name=f"ex_{i}")
                s_ = tp.tile([P, 1], f, tag="sm1", name=f"s_{i}")
                nc.scalar.activation(ex, lg, AF.Exp, accum_out=s_)
                nc.vector.reciprocal(s_, s_)
                nc.vector.tensor_scalar_mul(ex, ex, s_)
                tps = P2.tile([P, 512], f, tag="p2", name=f"tps_{i}")
                nc.tensor.transpose(tps[:O, :P], ex, ident)
                nc.vector.tensor_copy(out=opwT[:, ts(i, P)], in_=tps[:O, :P])
            nc.sync.dma_start(opw_dram[:], opwT[:])

            # ---------- phase 2: MoE (6 experts, PSUM-accumulated) ----------
            for t in range(TC):
                acc01 = P1.tile([P, 2, 512], f, tag="p1", name=f"acc01_{t}")
                acc23 = P1.tile([P, 2, 512], f, tag="p1", name=f"acc23_{t}")
                accs = [acc01[:, 0, :], acc01[:, 1, :],
                        acc23[:, 0, :], acc23[:, 1, :]]
                # group-of-GB software pipeline to batch the Sqrt table loads
                for g0 in range(0, O, GB):
                    grp = list(range(g0, min(g0 + GB, O)))
                    pres, rvs, wrs = {}, {}, {}
                    for o in grp:
                        wr = wrpp.tile([P, 512], BF16, tag="wrp",
                                       name=f"wr_{o}_{t}")
                        nc.sync.dma_start(
                            wr,
                            bass.AP(tensor=opw_dram, offset=o * T + t * 512,
                                    ap=[[0, P], [1, 512]]),
                        )
                        wrs[o] = wr
                        pre = prp.tile([P, C, 512], BF16, tag="pre",
                                       name=f"pre_{o}_{t}")
                        for m in range(C):
                            ps = P2.tile([P, 512], f, tag="p2",
                                         name=f"g1ps_{o}_{t}_{m}")
                            dr_gemm(ps, w1s[o], xT8, m, ts(t, 512))
                            if m % 2 == 0:
                                nc.vector.tensor_scalar_add(
                                    pre[:, m, :], ps, c1s[:, o, m : m + 1]
                                )
                            else:
                                nc.scalar.activation(
                                    pre[:, m, :], ps, AF.Identity,
                                    bias=c1s[:, o, m : m + 1], scale=1.0,
                                )
                        pres[o] = pre
                        # stats: mean and E[x^2] via 1/H-matmul accumulation
                        psm = P3.tile([P, 512], f, tag="p3",
                                      name=f"psm_{o}_{t}")
                        for c in range(C):
                            nc.tensor.matmul(psm, onesh, pre[:, c, :],
                                             start=(c == 0), stop=(c == C - 1))
                        psq = P3.tile([P, 512], f, tag="p3",
                                      name=f"psq_{o}_{t}")
                        for c in range(C):
                            sq = tp.tile([P, 512], BF16, tag="sq",
                                         name=f"sq_{o}_{t}_{c}")
                            if c % 2 == 0:
                                nc.scalar.activation(sq, pre[:, c, :],
                                                     AF.Square)
                            else:
                                nc.gpsimd.tensor_tensor(
                                    sq, pre[:, c, :], pre[:, c, :], ALU.mult
                                )
                            nc.tensor.matmul(psq, onesh, sq,
                                             start=(c == 0), stop=(c == C - 1))
                        mean = tp.tile([P, 512], BF16, tag="mean", bufs=3,
                                       name=f"mean_{o}_{t}")
                        nc.scalar.activation(mean, psm, AF.Copy)
                        sqm = tp.tile([P, 512], BF16, tag="sq2",
                                      name=f"sqm_{o}_{t}")
                        nc.scalar.activation(sqm, psq, AF.Copy)
                        m2v = tp.tile([P, 512], BF16, tag="lnstat_f",
                                      name=f"m2v_{o}_{t}")
                        nc.vector.tensor_tensor(m2v, mean, mean, ALU.mult)
                        nc.vector.scalar_tensor_tensor(
                            m2v, sqm, EPS, m2v, ALU.add, ALU.subtract,
                        )
                        rv = tp.tile([P, 512], f, tag="rvf", bufs=3,
                                     name=f"rv_{o}_{t}")
                        nc.vector.reciprocal(rv, m2v)
                        rvs[o] = (mean, rv)
                    # batched Sqrt ops (one act-table load per group)
                    rstds = {}
                    for o in grp:
                        rstd = tp.tile([P, 512], BF16, tag="rstd", bufs=3,
                                       name=f"rstd_{o}_{t}")
                        nc.scalar.activation(rstd, rvs[o][1], AF.Sqrt)
                        rstds[o] = rstd
                    # normalize + gelu (batched) + weight + GEMM2
                    for o in grp:
                        pre = pres[o]
                        mean = rvs[o][0]
                        rstd = rstds[o]
                        h = hp.tile([P, C, 512], BF16, tag="h",
                                    name=f"h_{o}_{t}")
                        for m in range(C):
                            d1 = tp.tile([P, 512], BF16, tag="dn",
                                         name=f"d1_{o}_{t}_{m}")
                            nc.vector.tensor_tensor(
                                d1, pre[:, m, :], mean, ALU.subtract
                            )
                            nc.vector.tensor_tensor(d1, d1, rstd, ALU.mult)
                            nc.scalar.activation(
                                h[:, m, :], d1, AF.Gelu,
                                bias=lnbs[:, o, m : m + 1],
                                scale=lngs[:, o, m : m + 1],
                            )
                        h8 = hp.tile([P, C, 512], F8, tag="h8",
                                     name=f"h8_{o}_{t}")
                        for m in range(C):
                            eng = nc.vector if m % 2 == 0 else nc.gpsimd
                            eng.tensor_tensor(
                                h8[:, m, :], h[:, m, :], wrs[o], ALU.mult
                            )
                        for m in range(C):
                            dr_gemm(accs[m], w2s[o], h8, m, slice(None),
                                    start=(o == 0), stop=(o == O - 1))
                # enhanced = x + routed expert sum
                for m in range(C):
                    nc.vector.tensor_tensor(
                        enhT[:, m, ts(t, 512)], accs[m], xTb[:, m, ts(t, 512)],
                        ALU.add,
                    )
                for m in range(C):
                    nc.gpsimd.tensor_copy(
                        out=enh8[:, m, ts(t, 512)], in_=enhT[:, m, ts(t, 512)]
                    )

            # ---------- phase 3: recursive reasoning (3 depths) ----------
            for d in range(D):
                src8 = enh8 if d == 0 else rec8
                proj8 = p8p.tile([P, C, T], F8, tag="p8t", name=f"proj8_{d}")
                for m in range(C):
                    for t in range(TC):
                        ps = P2.tile([P, 512], f, tag="p2",
                                     name=f"pjps_{d}_{m}_{t}")
                        dr_gemm(ps, rpj, src8, m, ts(t, 512))
                        nc.scalar.activation(
                            proj8[:, m, ts(t, 512)], ps, AF.Identity,
                            bias=rpb[:, m : m + 1], scale=1.0,
                        )
                # V first (feeds all heads' remote half), then per-head
                # K chain -> Q -> attention, so exp(h) overlaps K/Q(h+1).
                for qtr in range(4):
                    vc = vcp.tile([P, 2, H], BF16, tag="vch",
                                  name=f"vc_{d}_{qtr}")
                    for ii in range(2):
                        i = qtr * 2 + ii
                        ps = P2.tile([P, 512], f, tag="p2",
                                     name=f"vps_{d}_{i}")
                        for kp in range(KP):
                            nc.tensor.matmul(
                                ps,
                                proj8[:, 2 * kp : 2 * kp + 2, ts(i, P)],
                                wv[:, 2 * kp : 2 * kp + 2, :],
                                start=(kp == 0), stop=(kp == KP - 1),
                                perf_mode=DR,
                            )
                        nc.vector.tensor_tensor(vc[:, ii, :], ps, ibv,
                                                ALU.add)
                        nc.gpsimd.tensor_copy(out=vloc8[:, i, :],
                                              in_=vc[:, ii, :])
                    nc.sync.dma_start(v_send[:, qtr * 2 : qtr * 2 + 2], vc)
                if sim_mode:
                    nc.sync.dma_start(v_sum[:], v_send[:])
                else:
                    nc.gpsimd.collective_compute(
                        "AllReduce", ALU.add, replica_groups=RG,
                        ins=[v_send[:]], outs=[v_sum[:]],
                    )
                for qtr in range(4):
                    sl = (slice(None), slice(qtr * 2, qtr * 2 + 2),
                          slice(None))
                    vsm = vcp.tile([P, 2, H], BF16, tag="vch2",
                                   name=f"vsm_{d}_{qtr}")
                    nc.sync.dma_start(vsm, v_sum[:, qtr * 2 : qtr * 2 + 2])
                    nc.gpsimd.tensor_tensor(vrem8[sl], vsm, vloc8[sl],
                                            ALU.subtract)
                def emit_k(h):
                    for t in range(TC):
                        ps = P1.tile([64, 2, 512], f, tag="p1",
                                     name=f"kps_{d}_{h}_{t}")
                        for s in range(2):
                            for kp in range(KP):
                                nc.tensor.matmul(
                                    ps[:, s, :],
                                    wk[:, 2 * kp : 2 * kp + 2,
                                       (h * 2 + s) * 64 : (h * 2 + s + 1) * 64],
                                    proj8[:, 2 * kp : 2 * kp + 2, ts(t, 512)],
                                    start=(kp == 0), stop=(kp == KP - 1),
                                    perf_mode=DR,
                                )
                        kc = kcp.tile([64, 2, 512], BF16, tag="kch",
                                      name=f"kc_{d}_{h}_{t}")
                        for s in range(2):
                            nc.vector.tensor_scalar_add(
                                kc[:, s, :], ps[:, s, :], ibk[:, h, s : s + 1]
                            )
                        nc.vector.tensor_copy(
                            out=kT8[:, h, :, ts(t, 512)], in_=kc
                        )
                        nc.sync.dma_start(k_send[h, :, :, ts(t, 512)], kc)
                    if sim_mode:
                        nc.sync.dma_start(k_sum[h], k_send[h])
                    else:
                        nc.gpsimd.collective_compute(
                            "AllReduce", ALU.add, replica_groups=RG,
                            ins=[k_send[h]], outs=[k_sum[h]],
                        )
                    for t in range(TC):
                        ksm = kcp.tile([64, 2, 512], BF16, tag="ksm",
                                       name=f"ksm_{d}_{h}_{t}")
                        nc.sync.dma_start(ksm, k_sum[h, :, :, ts(t, 512)])
                        nc.gpsimd.tensor_tensor(
                            krem8[:, h, :, ts(t, 512)], ksm,
                            kT8[:, h, :, ts(t, 512)], ALU.subtract,
                        )

                def emit_q(h):
                    for t in range(TC):
                        ps = P1.tile([64, 2, 512], f, tag="p1",
                                     name=f"qps_{d}_{h}_{t}")
                        for s in range(2):
                            for kp in range(KP):
                                nc.tensor.matmul(
                                    ps[:, s, :],
                                    wq[:, 2 * kp : 2 * kp + 2,
                                       (h * 2 + s) * 64 : (h * 2 + s + 1) * 64],
                                    proj8[:, 2 * kp : 2 * kp + 2, ts(t, 512)],
                                    start=(kp == 0), stop=(kp == KP - 1),
                                    perf_mode=DR,
                                )
                        for s in range(2):
                            nc.scalar.activation(
                                qT8[:, h, s, ts(t, 512)], ps[:, s, :],
                                AF.Identity, bias=ibq[:, h, s : s + 1],
                                scale=1.0,
                            )

                def emit_attn(h):
                    for qc in range(TC):
                        dacc = P2.tile([P, 512], f, tag="p2",
                                       name=f"dacc_{d}_{h}_{qc}")
                        cctx = P3.tile([P, 512], f, tag="p3",
                                       name=f"cctx_{d}_{h}_{qc}")
                        for j in range(NPAIR):
                            sp = P1.tile([P, 2, 512], f, tag="p1",
                                         name=f"sp_{d}_{h}_{qc}_{j}")
                            for u in range(2):
                                kt = 2 * j + u
                                if kt < KT // 2:
                                    klhs = kT8[:, h, :, ts(kt, P)]
                                else:
                                    klhs = krem8[:, h, :, ts(kt - KT // 2, P)]
                                nc.tensor.matmul(
                                    sp[:, u, :], klhs,
                                    qT8[:, h, :, ts(qc, 512)],
                                    start=True, stop=True, perf_mode=DR,
                                )
                            ex8 = xp.tile([P, 2, 512], F8, tag="exl",
                                          name=f"ex_{d}_{h}_{qc}_{j}")
                            nc.scalar.activation(ex8, sp, AF.Exp,
                                                 scale=scale_s)
                            if j < NPAIR // 2:
                                vsel = vloc8[:, 2 * j : 2 * j + 2, ts(h, P)]
                            else:
                                jj = 2 * j - TT
                                vsel = vrem8[:, jj : jj + 2, ts(h, P)]
                            nc.tensor.matmul(dacc, ones8, ex8,
                                             start=(j == 0),
                                             stop=(j == NPAIR - 1),
                                             perf_mode=DR)
                            nc.tensor.matmul(cctx, vsel, ex8,
                                             start=(j == 0),
                                             stop=(j == NPAIR - 1),
                                             perf_mode=DR)
                        rd = tp.tile([P, 512], f, tag="rvf", bufs=3,
                                     name=f"rd_{d}_{h}_{qc}")
                        nc.vector.reciprocal(rd, dacc)
                        nc.vector.tensor_tensor(
                            ctx8[:, h, ts(qc, 512)], cctx, rd, ALU.mult
                        )

                # two-head waves: K/Q of the wave first (collectives in
                # flight), then attention of the wave overlapping the next
                # wave's K/Q.
                emit_k(0)
                emit_q(0)
                emit_k(1)
                emit_q(1)
                emit_attn(0)
                emit_k(2)
                emit_q(2)
                emit_attn(1)
                emit_k(3)
                emit_q(3)
                emit_attn(2)
                emit_attn(3)
                # out-proj + aggregate
                att8 = p8p.tile([P, C, T], F8, tag="p8t", name=f"att8_{d}")
                for m in range(C):
                    for t in range(TC):
                        ps = P2.tile([P, 512], f, tag="p2",
                                     name=f"ops_{d}_{m}_{t}")
                        dr_gemm(ps, ow, ctx8, m, ts(t, 512))
                        nc.vector.tensor_scalar_add(
                            att8[:, m, ts(t, 512)], ps, oba[:, m : m + 1]
                        )
                for m in range(C):
                    for t in range(TC):
                        ps = P3.tile([P, 512], f, tag="p3",
                                     name=f"agps_{d}_{m}_{t}")
                        dr_gemm(ps, wa, att8, m, ts(t, 512))
                        nc.vector.tensor_scalar_add(
                            rec8[:, m, ts(t, 512)], ps, dcs[:, d, m : m + 1]
                        )
                sc = 0.5 ** (d + 1)
                for m in range(C):
                    for t in range(TC):
                        nc.vector.scalar_tensor_tensor(
                            enhT[:, m, ts(t, 512)], rec8[:, m, ts(t, 512)],
                            sc, enhT[:, m, ts(t, 512)], ALU.mult, ALU.add,
                        )

            # ---------- phase 4: gating ----------
            for m in range(C):
                for t in range(TC):
                    nc.gpsimd.tensor_copy(
                        out=enh8[:, m, ts(t, 512)], in_=enhT[:, m, ts(t, 512)]
                    )
            gateT = bigp.tile([P, C, T], BF16, tag="biga", name="gateT")
            for t in range(TC):
                for mp_ in range(2):
                    gps = P1.tile([P, 2, 512], f, tag="p1",
                                  name=f"gps_{t}_{mp_}")
                    for u in range(2):
                        m = 2 * mp_ + u
                        for kp in range(KP):
                            nc.tensor.matmul(
                                gps[:, u, :],
                                gw1[:, 2 * kp : 2 * kp + 2, ts(m, P)],
                                xT8[:, 2 * kp : 2 * kp + 2, ts(t, 512)],
                                start=(kp == 0), stop=False, perf_mode=DR,
                            )
                        for kp in range(KP):
                            nc.tensor.matmul(
                                gps[:, u, :],
                                gw2[:, 2 * kp : 2 * kp + 2, ts(m, P)],
                                enh8[:, 2 * kp : 2 * kp + 2, ts(t, 512)],
                                start=False, stop=(kp == KP - 1),
                                perf_mode=DR,
                            )
                    # one wide sigmoid per m-pair; bias varies per m chunk so
                    # add it with the per-partition scalar slots of the two
                    # halves separately only if needed (biases share partition
                    # layout across the pair -> same [P,1] slice per half).
                    for u in range(2):
                        m = 2 * mp_ + u
                        nc.scalar.activation(
                            gateT[:, m, ts(t, 512)], gps[:, u, :], AF.Sigmoid,
                            bias=gbs[:, m : m + 1], scale=1.0,
                        )
            for m in range(C):
                for t in range(TC):
                    sl = (slice(None), m, ts(t, 512))
                    dt_ = tp.tile([P, 512], BF16, tag="t512",
                                  name=f"gd_{m}_{t}")
                    nc.vector.tensor_tensor(dt_, enhT[sl], xTb[sl],
                                            ALU.subtract)
                    nc.vector.tensor_tensor(dt_, dt_, gateT[sl], ALU.mult)
                    nc.vector.tensor_tensor(enhT[sl], xTb[sl], dt_, ALU.add)

            # ---------- phase 5: output LayerNorm + final linear ----------
            lnT = bigp.tile([P, C, T], BF16, tag="biga", name="lnT")
            stats = {}
            for t in range(TC):
                psm = P2.tile([P, 512], f, tag="p2", name=f"fpsm_{t}")
                for c in range(C):
                    nc.tensor.matmul(psm, onesh, enhT[:, c, ts(t, 512)],
                                     start=(c == 0), stop=(c == C - 1))
                psq = P3.tile([P, 512], f, tag="p3", name=f"fpsq_{t}")
                for c in range(C):
                    sq = tp.tile([P, 512], BF16, tag="sq",
                                 name=f"fsq_{t}_{c}")
                    nc.scalar.activation(sq, enhT[:, c, ts(t, 512)],
                                         AF.Square)
                    nc.tensor.matmul(psq, onesh, sq,
                                     start=(c == 0), stop=(c == C - 1))
                mean = tp.tile([P, 512], BF16, tag="mean", bufs=3, name=f"fmean_{t}")
                nc.vector.tensor_copy(out=mean, in_=psm)
                sqm = tp.tile([P, 512], BF16, tag="sq2", name=f"fsqm_{t}")
                nc.scalar.activation(sqm, psq, AF.Copy)
                m2v = tp.tile([P, 512], BF16, tag="lnstat_f", name=f"fm2v_{t}")
                nc.vector.tensor_tensor(m2v, mean, mean, ALU.mult)
                nc.vector.scalar_tensor_tensor(
                    m2v, sqm, EPS, m2v, ALU.add, ALU.subtract,
                )
                rv = tp.tile([P, 512], f, tag="rvf", bufs=3, name=f"frv_{t}")
                nc.vector.reciprocal(rv, m2v)
                stats[t] = (mean, rv)
            for t in range(TC):
                mean, rv = stats[t]
                rstd = tp.tile([P, 512], BF16, tag="rstd", bufs=3, name=f"frstd_{t}")
                nc.scalar.activation(rstd, rv, AF.Sqrt)
                stats[t] = (mean, rstd)
            for t in range(TC):
                mean, rstd = stats[t]
                for m in range(C):
                    sl = (slice(None), m, ts(t, 512))
                    d1 = tp.tile([P, 512], BF16, tag="dn",
                                 name=f"fd1_{t}_{m}")
                    nc.vector.tensor_tensor(d1, enhT[sl], mean, ALU.subtract)
                    nc.vector.tensor_tensor(d1, d1, rstd, ALU.mult)
                    nc.vector.tensor_scalar(
                        lnT[sl], d1, og[:, m : m + 1], ob[:, m : m + 1],
                        ALU.mult, ALU.add,
                    )
                for i in range(t * TT // TC, (t + 1) * TT // TC):
                    ps = P2.tile([P, 512], f, tag="p2", name=f"fps_{i}")
                    for k in range(C):
                        nc.tensor.matmul(
                            ps, lnT[:, k, ts(i, P)], wf[:, k, :],
                            start=(k == 0), stop=(k == C - 1),
                        )
                    ot = tp.tile([P, 512], f, tag="t512", name=f"ot_{i}")
                    nc.vector.tensor_tensor(ot, ps, obF, ALU.add)
                    nc.sync.dma_start(out_d[ts(i, P), :], ot)

    nc.compile()
    return nc


# ---------------------------------------------------------------------------
# host side
# ---------------------------------------------------------------------------

FP8 = ml_dtypes.float8_e4m3
BFH = ml_dtypes.bfloat16


def _lhsT(w):
    """w [fout, fin] (y = x @ w.T) -> stationary layout [P, fin//P, fout]."""
    wt = np.ascontiguousarray(np.asarray(w, np.float32).T)
    fi, fo = wt.shape
    return np.ascontiguousarray(wt.reshape(fi // P, P, fo).transpose(1, 0, 2))


def _fvec(v, nch=None):
    v = np.asarray(v, np.float32)
    n = v.shape[-1] // P if nch is None else nch
    return np.ascontiguousarray(v.reshape(n, P).T)


def _prep_weights(i):
    w = {}
    w["selw1"] = _lhsT(i["sel_W1"]).astype(FP8)
    w["sw2"] = _lhsT(i["sel_W2"]).astype(FP8)
    w["selb1"] = _fvec(i["sel_b1"])
    w["selb2"] = np.asarray(i["sel_b2"], np.float32)
    w["w1x"] = np.stack(
        [_lhsT(i["op_W1"][o, :, :H]) for o in range(O)]
    ).astype(FP8)
    w["w2"] = np.stack([_lhsT(i["op_W2"][o]) for o in range(O)]).astype(FP8)
    c1 = np.stack(
        [i["op_emb"][o] @ i["op_W1"][o, :, H:].T + i["op_b1"][o]
         for o in range(O)]
    ).astype(np.float32)
    w["c1"] = np.ascontiguousarray(
        np.stack([_fvec(c1[o]) for o in range(O)]).transpose(1, 0, 2)
    )
    w["lng"] = np.ascontiguousarray(
        np.stack([_fvec(i["op_ln_g"][o]) for o in range(O)]).transpose(1, 0, 2)
    )
    w["lnb"] = np.ascontiguousarray(
        np.stack([_fvec(i["op_ln_b"][o]) for o in range(O)]).transpose(1, 0, 2)
    )
    w["wq"] = _lhsT(i["attn_in_w"][:H]).astype(FP8)
    w["wk"] = _lhsT(i["attn_in_w"][H : 2 * H]).astype(FP8)
    w["wv"] = _lhsT(i["attn_in_w"][2 * H :]).astype(FP8)
    w["ibq"] = np.ascontiguousarray(
        np.asarray(i["attn_in_b"][:H], np.float32)
        .reshape(NH, 2, 64).transpose(2, 0, 1)
    )
    w["ibk"] = np.ascontiguousarray(
        np.asarray(i["attn_in_b"][H : 2 * H], np.float32)
        .reshape(NH, 2, 64).transpose(2, 0, 1)
    )
    w["ibv"] = np.asarray(i["attn_in_b"][2 * H :], np.float32)
    w["ow"] = _lhsT(i["attn_out_w"]).astype(FP8)
    w["oba"] = _fvec(i["attn_out_b"])
    w["rpj"] = _lhsT(i["rec_proj_w"]).astype(FP8)
    w["rpb"] = _fvec(i["rec_proj_b"])
    w["wa"] = _lhsT(i["rec_agg_w"][:, :H]).astype(FP8)
    dc = np.stack(
        [i["depth_emb"][d] @ i["rec_agg_w"][:, H:].T + i["rec_agg_b"]
         for d in range(D)]
    ).astype(np.float32)
    w["dc"] = np.ascontiguousarray(
        np.stack([_fvec(dc[d]) for d in range(D)]).transpose(1, 0, 2)
    )
    w["gw1"] = _lhsT(i["gate_w"][:, :H]).astype(FP8)
    w["gw2"] = _lhsT(i["gate_w"][:, H:]).astype(FP8)
    w["gateb"] = _fvec(i["gate_b"])
    w["olng"] = _fvec(i["out_ln_g"])
    w["olnb"] = _fvec(i["out_ln_b"])
    w["wf"] = _lhsT(i["out_w"]).astype(BFH)
    w["outbF"] = np.asarray(i["out_b"], np.float32)
    w["onesh_in"] = np.full((P, P), 1.0 / H, BFH)
    w["ones8_in"] = np.ones((P, 2, P), FP8)
    return w


def make_in_maps(inputs):
    inputs = {k: np.asarray(v, np.float32) for k, v in inputs.items()}
    hs = inputs["hidden_states"]
    w = _prep_weights(inputs)
    in_maps = []
    for c in range(8):
        b, half = c // 2, c % 2
        m = dict(w)
        x = hs[b, half * T : (half + 1) * T, :]       # [T, H]
        xt = np.ascontiguousarray(
            x.T.reshape(C, P, T).transpose(1, 0, 2)   # [P, C, T]
        )
        m["xT8"] = xt.astype(FP8)
        m["xTb"] = xt.astype(BFH)
        in_maps.append(m)
    return in_maps


def assemble_out(results):
    out = np.empty((4, S, H), np.float32)
    for c in range(8):
        b, half = c // 2, c % 2
        out[b, half * T : (half + 1) * T, :] = results[c]["out"]
    return out


def kernel(**inputs):
    in_maps = make_in_maps(inputs)
    if "nc" not in _CACHE:
        _CACHE["nc"] = build_bass()
    res = run_bass_kernel_spmd(nc=_CACHE["nc"], in_maps=in_maps,
                               core_ids=list(range(8)))
    return assemble_out(res.results)


if __name__ == "__main__":
    print("build-only smoke test")
    build_bass()
    print("ok")
